# revision 7
# baseline (speedup 1.0000x reference)
"""Trainium2 Bass kernel for a dense recurrent scan (nn_CXBPU_55611236549128).

Math (per timestep t, K=4 microsteps):
    inj  = x_t @ W_in.T + b_in                  scattered into sensory_indices
    h    = relu(h @ W_rec.T + scatter(inj))     microstep 0
    h    = relu(h @ W_rec.T)                    microsteps 1..K-1
    out_t = h[:, output_indices] @ W_out.T + b_out

Sharding: data-parallel over batch, 8 rows per core, W_rec replicated.

Per-core design (feature-major "hT" layout [128 partitions, 16 chunks x 8 batch]):
  - Single-pass fp16 matmuls, h-stationary: W_rec.T resident in SBUF and
    streamed as the moving operand (the fast streaming port), hT chunks as
    the 8-column stationary.  End-to-end error vs the fp32 reference is
    ~8e-4 (the recurrence is contractive, so per-step fp16 rounding damps).
  - 4 PE column groups (tile_position=(0,32j)) each stream their own
    k-tiles; rounds of 4 concurrent matmuls pipeline at the 512-col
    streaming cadence (~216 ns).
  - PSUM layout: 4 separate one-bank tiles for the 4 output-column banks
    plus 4 separate one-bank psumT tiles (one per k-group).  Separate
    tiles per bank are essential: a single multi-bank tile makes Tile's
    PSUM tracker serialize every bank's first matmul behind the previous
    bank's evacuation read (~1 us stall per bank).
  - Tail per bank n: two half-bank casts (DVE + ACT in parallel) fp32->fp16
    into batch-major evac, then 4 "transpose-sum" matmuls against a 0/1
    selector (i128) fold the 4 partition groups into feature-major psumT_n,
    then one DVE relu produces the hT chunk group.  Bank 3's transpose-sum
    + relu are deferred into the next microstep's instruction stream.
  - Injection is one extra tiny matmul per bank on microstep 0:
    lhsT = [x_t^T; 1] (8 partitions), rhs = scatter-expanded W_in/b_in.
  - Readout: 4 column-split accumulation chains over hT chunks, partials
    folded with one selector matmul, ACT-copied to an SBUF staging tile.
"""

import os
from contextlib import ExitStack

import numpy as np

N = 2048
B = 64
T = 128
NCORES = 8
BPC = B // NCORES  # 8 batch rows per core
NCHUNK = N // 128  # 16

_CACHE = {}


def _build_nc(n_steps):
    import concourse.bass as bass
    import concourse.mybir as mybir
    import concourse.tile as tile
    from concourse import bacc

    f32 = mybir.dt.float32
    f16 = mybir.dt.float16
    nc = bacc.Bacc(trn_type="TRN2")

    wt_d = nc.dram_tensor("wt", [N, N], f16, kind="ExternalInput")
    winj_d = nc.dram_tensor("winj", [8, N], f16, kind="ExternalInput")
    xt_d = nc.dram_tensor("xt", [8, n_steps * BPC], f16, kind="ExternalInput")
    wsel_d = nc.dram_tensor("wsel", [128, 2 * NCHUNK], f16, kind="ExternalInput")
    i128_d = nc.dram_tensor("i128", [128, BPC], f16, kind="ExternalInput")
    out_d = nc.dram_tensor("out", [2, n_steps * BPC], f32, kind="ExternalOutput")

    with tile.TileContext(nc) as tc, ExitStack() as ctx:
        const = ctx.enter_context(tc.tile_pool(name="const", bufs=1))
        hpool = ctx.enter_context(tc.tile_pool(name="h", bufs=3))
        epool = ctx.enter_context(tc.tile_pool(name="evac", bufs=3))
        rpool = ctx.enter_context(tc.tile_pool(name="prs", bufs=2))
        ppool = ctx.enter_context(tc.tile_pool(name="psum", bufs=1, space="PSUM"))

        # resident W^T slabs: slab kk (k-tile) at cols [kk*N, (kk+1)*N).
        wt = const.tile([128, NCHUNK * N], f16)
        for u in range(NCHUNK):
            eng = (nc.sync, nc.scalar, nc.gpsimd)[u % 3]
            eng.dma_start(wt[:, u * N : (u + 1) * N], wt_d[u * 128 : (u + 1) * 128, :])
        winj = const.tile([8, N], f16)
        nc.sync.dma_start(winj[:], winj_d[:])
        xt = const.tile([8, n_steps * BPC], f16)
        nc.scalar.dma_start(xt[:], xt_d[:])
        wsel = const.tile([128, 2 * NCHUNK], f16)
        nc.gpsimd.dma_start(wsel[:], wsel_d[:])
        i128 = const.tile([128, BPC], f16)
        nc.sync.dma_start(i128[:], i128_d[:])
        outst = const.tile([2, n_steps * BPC], f32)

        # PSUM: exactly 8 banks.  pbank[n] = output cols [512n, 512n+512);
        # psumT[n] = feature-major chunk group n (cols 0:32 used; the
        # remainder of banks 6/7 hosts the readout partials/sum).
        pbank = [ppool.tile([128, 512], f32, name=f"pbank{n}") for n in range(4)]
        psumT = [ppool.tile([128, 512], f32, name=f"psumT{n}") for n in range(4)]
        PR = 448  # col offset of readout scratch inside psumT[3]/psumT[2]

        # readout partial region read by a [128 x 8] DVE cast; zero the
        # never-written partitions once so the selector's 0-weights don't
        # multiply uninitialized PSUM.
        nc.vector.memset(psumT[3][:, PR : PR + BPC], 0.0)

        hT = hpool.tile([128, NCHUNK * BPC], f16)
        nc.vector.memset(hT[:], 0.0)

        tc.strict_bb_all_engine_barrier()

        # Bank 3's transpose-sum + relu (and the per-timestep readout) are
        # deferred into the NEXT microstep's instruction stream; the next
        # microstep's round r only reads relu-group r, so group 3 is needed
        # only ~650 ns in.
        pending = []

        for t in range(n_steps):
            for s in range(4):
                evac = epool.tile([128, N], f16)
                hT_new = hpool.tile([128, NCHUNK * BPC], f16)

                def main_bank(n, s=s, t=t, hT=hT):
                    # psum[32j+b, :] += sum_k h[b,k] Wrec[512n+c,k]; col group
                    # j handles k-tiles {4r+j}, j innermost so the 4 streams
                    # overlap; contraction split over groups is folded later
                    # by the transpose-sum.
                    for r in range(4):
                        for j in range(4):
                            kk = 4 * r + j
                            nc.tensor.matmul(
                                pbank[n][32 * j : 32 * j + BPC, :],
                                lhsT=hT[:, kk * BPC : (kk + 1) * BPC],
                                rhs=wt[:, kk * N + 512 * n : kk * N + 512 * (n + 1)],
                                start=(r == 0),
                                stop=(r == 3 and s != 0),
                                tile_position=(0, 32 * j),
                            )
                    if s == 0:
                        # injection fused into the accumulation: one tiny
                        # matmul, lhsT = [x_t^T; 1; 0] (8 partitions),
                        # rhs = scatter-expanded [W_in; b_in] columns.
                        nc.tensor.matmul(
                            pbank[n][0:BPC, :],
                            lhsT=xt[:, t * BPC : (t + 1) * BPC],
                            rhs=winj[:, 512 * n : 512 * (n + 1)],
                            start=False,
                            stop=True,
                        )

                def cast_bank(n, evac=evac):
                    # fp32 psum -> fp16 batch-major evac, halves on DVE and
                    # ACT in parallel to shorten the tail latency.
                    nc.vector.tensor_copy(
                        evac[:, 512 * n : 512 * n + 256], pbank[n][:, 0:256]
                    )
                    nc.scalar.copy(
                        evac[:, 512 * n + 256 : 512 * n + 512], pbank[n][:, 256:512]
                    )

                def tmm_group(n, evac=evac):
                    # transpose-sum: psumT_n[m, ci*8+b] = sum_j evac[32j+b, .]
                    # split into col groups ([128,32] stationary at strip q)
                    # so the 16 matmuls ride inside the 4 column-group
                    # streams instead of draining the full array.
                    for ci in range(4):
                        c = 4 * n + ci
                        for q in range(4):
                            nc.tensor.matmul(
                                psumT[n][32 * q : 32 * q + 32,
                                         ci * BPC : (ci + 1) * BPC],
                                lhsT=evac[:, c * 128 + 32 * q : c * 128 + 32 * q + 32],
                                rhs=i128[:],
                                start=True,
                                stop=True,
                                tile_position=(0, 32 * q),
                            )

                def relu_group(n, hT_new=hT_new):
                    nc.vector.tensor_relu(
                        hT_new[:, 32 * n : 32 * n + 32], psumT[n][:, 0:32]
                    )

                # flush deferred tail of the previous microstep first
                for fn in pending:
                    fn()
                pending = []

                # relu_g(n) is issued BEFORE cast(n+1) so the DVE's strict
                # FIFO doesn't park the relu behind a cast that waits on the
                # next bank's stop.
                main_bank(0)
                cast_bank(0)
                main_bank(1)
                tmm_group(0)
                relu_group(0)
                cast_bank(1)
                main_bank(2)
                tmm_group(1)
                relu_group(1)
                cast_bank(2)
                main_bank(3)
                tmm_group(2)
                relu_group(2)
                cast_bank(3)
                pending = [
                    lambda n=3, f=tmm_group: f(n),
                    lambda n=3, f=relu_group: f(n),
                ]

                if s == 3:
                    def readout(t=t, hT_new=hT_new):
                        # 16-chunk accumulation chain (pipelines at the
                        # 8-col issue rate), then ACT-copied out.
                        for c in range(NCHUNK):
                            nc.tensor.matmul(
                                psumT[2][0:2, PR : PR + BPC],
                                lhsT=wsel[:, c * 2 : (c + 1) * 2],
                                rhs=hT_new[:, c * BPC : (c + 1) * BPC],
                                start=(c == 0),
                                stop=(c == NCHUNK - 1),
                            )
                        nc.scalar.copy(
                            outst[:, t * BPC : (t + 1) * BPC],
                            psumT[2][0:2, PR : PR + BPC],
                        )

                    pending.append(readout)

                hT = hT_new

        for fn in pending:
            fn()
        nc.sync.dma_start(out_d[:], outst[:])
    nc.compile()
    return nc


def _prep_inputs(inputs, W_rec, W_in, b_in, W_out, sensory_indices, output_indices,
                 n_steps):
    inputs = np.asarray(inputs, np.float32)
    W_rec = np.asarray(W_rec, np.float32)
    W_in = np.asarray(W_in, np.float32)
    b_in = np.asarray(b_in, np.float32)
    W_out = np.asarray(W_out, np.float32)
    sens = np.asarray(sensory_indices).astype(np.int64)
    oidx = np.asarray(output_indices).astype(np.int64)

    wt = np.ascontiguousarray(W_rec.T).astype(np.float16)

    # scatter-expanded readout weights, feature-major by chunk
    wsel_full = np.zeros((2, N), np.float32)
    np.add.at(wsel_full, (slice(None), oidx), W_out)
    wsel = np.ascontiguousarray(
        wsel_full.reshape(2, NCHUNK, 128).transpose(2, 1, 0).reshape(128, 2 * NCHUNK)
    ).astype(np.float16)

    i128 = (np.arange(128)[:, None] % 32 == np.arange(BPC)[None, :]).astype(np.float16)

    # scatter-expanded injection weights: rows 0-3 = W_in.T, row 4 = b_in
    winj_full = np.zeros((8, N), np.float32)
    np.add.at(winj_full, (slice(None, 4), sens), W_in.T)
    np.add.at(winj_full[4], sens, b_in)
    winj = winj_full.astype(np.float16)

    # per-core x_t stationary blocks: [8, T*8], rows 0-3 = x_t^T, row 4 = 1
    ncin = inputs.shape[2]
    xt_cores = []
    for g in range(NCORES):
        a = inputs[g * BPC : (g + 1) * BPC, :n_steps, :]  # [8, T, 4]
        x = np.zeros((8, n_steps, BPC), np.float32)
        x[:ncin] = a.transpose(2, 1, 0)
        x[4] = 1.0
        xt_cores.append(np.ascontiguousarray(x.reshape(8, n_steps * BPC)).astype(np.float16))

    return wt, winj, xt_cores, wsel, i128


def _run(inputs, W_rec, W_in, b_in, W_out, b_out, sensory_indices, output_indices,
         K, n_steps=T, trace=False):
    from concourse.bass_utils import run_bass_kernel_spmd

    assert int(K) == 4
    wt, winj, xt_cores, wsel, i128 = _prep_inputs(
        inputs, W_rec, W_in, b_in, W_out, sensory_indices, output_indices, n_steps)

    if n_steps not in _CACHE:
        _CACHE[n_steps] = _build_nc(n_steps)
    nc = _CACHE[n_steps]

    in_maps = [
        {"wt": wt, "winj": winj, "xt": xt_cores[g], "wsel": wsel, "i128": i128}
        for g in range(NCORES)
    ]
    res = run_bass_kernel_spmd(nc, in_maps, list(range(NCORES)), trace=trace)

    b_out = np.asarray(b_out, np.float32)
    outs = []
    for g in range(NCORES):
        r = np.asarray(res.results[g]["out"])  # [2, T*8]
        outs.append(r.reshape(2, n_steps, BPC).transpose(2, 1, 0))  # [8, T, 2]
    full = np.concatenate(outs, axis=0) + b_out  # [B, T, 2]
    return np.ascontiguousarray(full.astype(np.float32)), res


def kernel(**inputs):
    out, _ = _run(
        inputs["inputs"], inputs["W_rec"], inputs["W_in"], inputs["b_in"],
        inputs["W_out"], inputs["b_out"], inputs["sensory_indices"],
        inputs["output_indices"], inputs["K"],
    )
    return out
